# revision 5
# baseline (speedup 1.0000x reference)
"""GQA attention block (QKV proj + causal attention + output proj) on 8 trn2 cores.

Sharding: core c -> (batch b = c//4, kv-group g = c%4). Each core computes 4 Q
heads (one KV-head group) of one batch and a partial o_proj output; the host
sums the 4 partials per batch (row-sharded o_proj all-reduce done host-side).

All matmul operands are bf16 (PE runs 1 cycle/row vs 4 for fp32) with fp32
PSUM accumulation. Attention uses transposed scores S^T[tk, tq] so the softmax
denominator comes for free from a ones-column appended to V, and no on-chip
transposes of attention weights are needed.

_build_nc(loop_reps=N) wraps the body in a For_i so one NEFF execution runs
the kernel N times — used by the bench harness to difference two rep counts
and cancel per-call RPC overhead (axon has no NTFF profiling).
"""

import math

import numpy as np

# Model dims (hardcoded per contract; kernel.py must be self-contained).
B = 2
T = 2048
E = 2048
HD = 128               # head dim
NH = 16                # query heads total
NKV = 4                # kv heads total
NHC = 4                # query heads per core
P = 128
KO = E // P            # 16 contraction subtiles of 128
TQC = T // 512         # 4 query chunks of 512
TB = T // P            # 16 t blocks of 128
SCALE = 1.0 / math.sqrt(HD)
N_CORES = 8

_NC_CACHE = {}


def _build_nc(loop_reps=1):
    import concourse.bacc as bacc
    import concourse.mybir as mybir
    import concourse.tile as tile
    from concourse.masks import make_identity, make_upper_triangular

    f32 = mybir.dt.float32
    bf16 = mybir.dt.bfloat16
    nc = bacc.Bacc(None, target_bir_lowering=False)

    xT = nc.dram_tensor("xT", [E, T], bf16, kind="ExternalInput")
    wqT = nc.dram_tensor("wqT", [E, NHC * HD], bf16, kind="ExternalInput")
    wkT = nc.dram_tensor("wkT", [E, HD], bf16, kind="ExternalInput")
    wvT = nc.dram_tensor("wvT", [E, HD], bf16, kind="ExternalInput")
    woT = nc.dram_tensor("woT", [NHC * HD, E], bf16, kind="ExternalInput")
    out = nc.dram_tensor("out", [T, E], f32, kind="ExternalOutput")

    xT_r = xT.rearrange("(ko p) t -> p ko t", p=P)        # [128, 16, T]
    wqT_r = wqT.rearrange("(ko p) d -> p ko d", p=P)      # [128, 16, 512]
    wkT_r = wkT.rearrange("(ko p) d -> p ko d", p=P)      # [128, 16, 128]
    wvT_r = wvT.rearrange("(ko p) d -> p ko d", p=P)
    woT_r = woT.rearrange("(h p) e -> p h e", p=P)        # [128, 4, E]
    out_r = out.rearrange("(tb p) e -> p tb e", p=P)      # [128, 16, E]

    with tile.TileContext(nc) as tc:
        with (
            tc.tile_pool(name="const", bufs=1) as constp,
            tc.tile_pool(name="qkv", bufs=1) as qkvp,
            tc.tile_pool(name="w1", bufs=1) as w1p,
            tc.tile_pool(name="xq", bufs=1) as xqp,
            tc.tile_pool(name="big2", bufs=1) as big2,
            tc.tile_pool(name="work", bufs=4) as work,
            tc.tile_pool(name="owork", bufs=3) as owork,
            tc.tile_pool(name="ps_acc", bufs=2, space="PSUM") as ps_acc,
            tc.tile_pool(name="ps_y", bufs=4, space="PSUM") as ps_y,
            tc.tile_pool(name="ps_t", bufs=2, space="PSUM") as ps_t,
        ):
            def body():
                identity = constp.tile([P, P], bf16, tag="ident")
                make_identity(nc, identity)

                # tri[p, q] = 1.0 where p <= q — causal mask for the one
                # tk==tq diagonal 128x128 sub-block.
                tri = constp.tile([P, P], bf16, tag="tri")
                make_upper_triangular(nc, tri[:], val=1.0, diag=True)

                QT = qkvp.tile([P, NHC, T], bf16, tag="QT")   # q^T per head [d, t]
                KT = qkvp.tile([P, T], bf16, tag="KT")        # k^T [d, t]
                VT = qkvp.tile([P, T], bf16, tag="VT")        # v^T [d, t]
                VAUG = qkvp.tile([P, TB, HD + 1], bf16, tag="VAUG")  # v [tk, 129]

                # ---- Phase 1: projections. q^T/k^T/v^T = W @ x^T, contracting
                # over e in one 16-matmul PSUM chain per output tile. K and V
                # first so attention (and the VAUG transposes) start early.
                XT = xqp.tile([P, KO, T], bf16, tag="XT")
                WQT = w1p.tile([P, KO, NHC * HD], bf16, tag="WQT")
                WKT = w1p.tile([P, KO, HD], bf16, tag="WKT")
                WVT = w1p.tile([P, KO, HD], bf16, tag="WVT")

                for ko in range(KO):
                    nc.sync.dma_start(XT[:, ko], xT_r[:, ko])
                    nc.sync.dma_start(WKT[:, ko], wkT_r[:, ko])
                    nc.sync.dma_start(WVT[:, ko], wvT_r[:, ko])
                    nc.sync.dma_start(WQT[:, ko], wqT_r[:, ko])

                def _acc(dst, lhsT_of_ko, tcol):
                    ps = ps_acc.tile([P, 512], f32, tag="ps_acc")
                    for ko in range(KO):
                        nc.tensor.matmul(
                            ps[:],
                            lhsT_of_ko(ko),
                            XT[:, ko, tcol * 512:(tcol + 1) * 512],
                            start=(ko == 0),
                            stop=(ko == KO - 1),
                        )
                    nc.vector.tensor_copy(dst, ps[:])

                for tcol in range(TQC):
                    _acc(KT[:, tcol * 512:(tcol + 1) * 512],
                         lambda ko: WKT[:, ko], tcol)
                for tcol in range(TQC):
                    _acc(VT[:, tcol * 512:(tcol + 1) * 512],
                         lambda ko: WVT[:, ko], tcol)
                for h in range(NHC):
                    for tcol in range(TQC):
                        _acc(
                            QT[:, h, tcol * 512:(tcol + 1) * 512],
                            lambda ko, h=h: WQT[:, ko, h * HD:(h + 1) * HD],
                            tcol,
                        )

                # v^T -> v natural layout blocks, with ones column for the
                # softmax denominator.
                nc.vector.memset(VAUG[:, :, HD:HD + 1], 1.0)
                for tb in range(TB):
                    pst = ps_t.tile([P, P], bf16, tag="ps_t")
                    nc.tensor.transpose(
                        pst[:], VT[:, tb * P:(tb + 1) * P], identity[:]
                    )
                    nc.vector.tensor_copy(VAUG[:, tb, 0:HD], pst[:])

                YT = big2.tile([P, NHC, T], bf16, tag="YT")  # y^T per head
                WOT = big2.tile([P, NHC, E], bf16, tag="WOT")
                for ko in range(4):
                    nc.sync.dma_start(WOT[:, ko], woT_r[:, ko])

                # ---- Phase 2: causal attention, transposed scores. For
                # diagonal-region tk blocks the score matmul is narrowed to
                # the causally-valid tq columns; only the single tk==tq
                # 128x128 sub-block needs the triangular mask.
                for h in range(NHC):
                    for tqc in range(TQC):
                        ntk = 4 * (tqc + 1)   # tk blocks up to the diagonal
                        psy = [
                            ps_y.tile([P, HD + 1], f32, tag="ps_y",
                                      name=f"psy_{j}")
                            for j in range(4)
                        ]
                        for tk in range(ntk):
                            i = tk - 4 * tqc  # >= 0 inside diagonal region
                            off = max(0, i) * P   # local tq offset of valid
                            w = 512 - off
                            pss = ps_acc.tile([P, 512], f32, tag="ps_acc")
                            nc.tensor.matmul(
                                pss[:, 0:w],
                                KT[:, tk * P:(tk + 1) * P],
                                QT[:, h, tqc * 512 + off:(tqc + 1) * 512],
                                start=True,
                                stop=True,
                            )
                            es = work.tile([P, 512], bf16, tag="expS")
                            nc.scalar.activation(
                                es[:, 0:w], pss[:, 0:w],
                                mybir.ActivationFunctionType.Exp,
                                scale=SCALE,
                            )
                            if i >= 0:
                                nc.vector.tensor_mul(
                                    out=es[:, 0:P], in0=es[:, 0:P], in1=tri[:]
                                )
                            for j in range(max(0, i), 4):
                                nc.tensor.matmul(
                                    psy[j][:],
                                    es[:, j * P - off:(j + 1) * P - off],
                                    VAUG[:, tk],
                                    start=(tk == 0),
                                    stop=(tk == 4 * tqc + j),
                                )
                        for j in range(4):
                            jg = 4 * tqc + j
                            recip = work.tile([P, 1], f32, tag="recip")
                            nc.vector.reciprocal(recip[:], psy[j][:, HD:HD + 1])
                            ysb = work.tile([P, P], bf16, tag="ysb")
                            nc.vector.tensor_scalar_mul(
                                ysb[:], psy[j][:, 0:HD], recip[:]
                            )
                            pst = ps_t.tile([P, P], bf16, tag="ps_t")
                            nc.tensor.transpose(pst[:], ysb[:], identity[:])
                            nc.vector.tensor_copy(
                                YT[:, h, jg * P:(jg + 1) * P], pst[:]
                            )

                # ---- Phase 3: o_proj partial: out = sum_h y_h^T.T @ woT_h
                for tb in range(TB):
                    for ec in range(4):
                        ps = ps_acc.tile([P, 512], f32, tag="ps_acc")
                        for h in range(NHC):
                            nc.tensor.matmul(
                                ps[:],
                                YT[:, h, tb * P:(tb + 1) * P],
                                WOT[:, h, ec * 512:(ec + 1) * 512],
                                start=(h == 0),
                                stop=(h == 3),
                            )
                        osb = owork.tile([P, 512], f32, tag="osb")
                        nc.vector.tensor_copy(osb[:], ps[:])
                        nc.sync.dma_start(
                            out_r[:, tb, ec * 512:(ec + 1) * 512], osb[:]
                        )

            if loop_reps == 1:
                body()
            else:
                with tc.For_i(0, loop_reps, 1):
                    body()

    nc.finalize()
    return nc


def _get_nc():
    if "nc" not in _NC_CACHE:
        _NC_CACHE["nc"] = _build_nc()
    return _NC_CACHE["nc"]


def _in_maps(x, wq, wk, wv, wo):
    import ml_dtypes

    bf16 = ml_dtypes.bfloat16
    xTb = [np.ascontiguousarray(x[b].T.astype(bf16)) for b in range(B)]
    wqT = [
        np.ascontiguousarray(wq[g * 512:(g + 1) * 512].T.astype(bf16))
        for g in range(NKV)
    ]
    wkT = [
        np.ascontiguousarray(wk[g * HD:(g + 1) * HD].T.astype(bf16))
        for g in range(NKV)
    ]
    wvT = [
        np.ascontiguousarray(wv[g * HD:(g + 1) * HD].T.astype(bf16))
        for g in range(NKV)
    ]
    woT = [
        np.ascontiguousarray(wo[:, g * 512:(g + 1) * 512].T.astype(bf16))
        for g in range(NKV)
    ]
    maps = []
    for c in range(N_CORES):
        b, g = divmod(c, NKV)
        maps.append({
            "xT": xTb[b],
            "wqT": wqT[g],
            "wkT": wkT[g],
            "wvT": wvT[g],
            "woT": woT[g],
        })
    return maps


def kernel(x, wq, wk, wv, wo):
    from concourse.bass_utils import run_bass_kernel_spmd

    x = np.asarray(x, dtype=np.float32)
    wq = np.asarray(wq, dtype=np.float32)
    wk = np.asarray(wk, dtype=np.float32)
    wv = np.asarray(wv, dtype=np.float32)
    wo = np.asarray(wo, dtype=np.float32)

    nc = _get_nc()
    in_maps = _in_maps(x, wq, wk, wv, wo)

    res = run_bass_kernel_spmd(nc, in_maps, core_ids=list(range(N_CORES)))

    partials = [res.results[c]["out"] for c in range(N_CORES)]
    out = np.empty((B, T, E), dtype=np.float32)
    for b in range(B):
        acc = partials[NKV * b].astype(np.float32)
        for g in range(1, NKV):
            acc = acc + partials[NKV * b + g]
        out[b] = acc
    return out


# revision 13
# speedup vs baseline: 2.9911x; 2.9911x over previous
"""GQA attention block (QKV proj + causal attention + output proj) on 8 trn2 cores.

Sharding: core c -> (batch b = c//4, kv-group g = c%4). Each core computes 4 Q
heads (one KV-head group) of one batch and a partial o_proj output; the host
sums the 4 partials per batch (row-sharded o_proj all-reduce done host-side).

All matmul operands are bf16 (PE streams ~2 cols/cycle vs 1 for fp32's
4-cycle rows) with fp32 PSUM accumulation. Attention uses transposed scores
S^T[tk, tq] so the softmax denominator comes for free from a ones-column
appended to V. Score blocks are computed in pairs into 2-bank [128,1024]
PSUM tiles so one Exp activation covers two blocks (halves ACT call count);
the four per-head psy accumulators live two-per-PSUM-bank, opened by a
zero-writing matmul whose start=True clears the whole bank's has_written
bits (values irrelevant: it writes zeros, later matmuls accumulate).
Q-projection is interleaved per head with that head's attention so ACT work
overlaps phase-1 PE work.

_build_nc(loop_reps=N) wraps the body in a For_i so one NEFF execution runs
the kernel N times — used by the bench harness to difference two rep counts
and cancel per-call RPC overhead (axon has no NTFF profiling).
"""

import math

import numpy as np

# Model dims (hardcoded per contract; kernel.py must be self-contained).
B = 2
T = 2048
E = 2048
HD = 128               # head dim
NH = 16                # query heads total
NKV = 4                # kv heads total
NHC = 4                # query heads per core
P = 128
KO = E // P            # 16 contraction subtiles of 128
TQC = T // 512         # 4 query chunks of 512
TB = T // P            # 16 t blocks of 128
SCALE = 1.0 / math.sqrt(HD)
N_CORES = 8

_NC_CACHE = {}


def _build_nc(loop_reps=1, phases=(1, 2, 3)):
    import concourse.bacc as bacc
    import concourse.mybir as mybir
    import concourse.tile as tile
    from concourse.masks import make_identity, make_upper_triangular

    f32 = mybir.dt.float32
    bf16 = mybir.dt.bfloat16
    nc = bacc.Bacc(None, target_bir_lowering=False)

    xT = nc.dram_tensor("xT", [E, T], bf16, kind="ExternalInput")
    wqT = nc.dram_tensor("wqT", [E, NHC * HD], bf16, kind="ExternalInput")
    wkT = nc.dram_tensor("wkT", [E, HD], bf16, kind="ExternalInput")
    wvT = nc.dram_tensor("wvT", [E, HD], bf16, kind="ExternalInput")
    woT = nc.dram_tensor("woT", [NHC * HD, E], bf16, kind="ExternalInput")
    out = nc.dram_tensor("out", [T, E], bf16, kind="ExternalOutput")

    xT_r = xT.rearrange("(ko p) t -> p ko t", p=P)        # [128, 16, T]
    wqT_r = wqT.rearrange("(ko p) d -> p ko d", p=P)      # [128, 16, 512]
    wkT_r = wkT.rearrange("(ko p) d -> p ko d", p=P)      # [128, 16, 128]
    wvT_r = wvT.rearrange("(ko p) d -> p ko d", p=P)
    woT_r = woT.rearrange("(h p) e -> p h e", p=P)        # [128, 4, E]
    out_r = out.rearrange("(tb p) e -> p tb e", p=P)      # [128, 16, E]

    with tile.TileContext(nc) as tc:
        with (
            tc.tile_pool(name="const", bufs=1) as constp,
            tc.tile_pool(name="qkv", bufs=1) as qkvp,
            tc.tile_pool(name="w1", bufs=1) as w1p,
            tc.tile_pool(name="xq", bufs=1) as xqp,
            tc.tile_pool(name="big2", bufs=1) as big2,
            tc.tile_pool(name="work", bufs=4) as work,
            tc.tile_pool(name="owork", bufs=3) as owork,
            tc.tile_pool(name="ps_acc", bufs=2, space="PSUM") as ps_acc,
            tc.tile_pool(name="ps_y", bufs=2, space="PSUM") as ps_y,
            tc.tile_pool(name="ps_t", bufs=2, space="PSUM") as ps_t,
        ):
            def body():
                identity = constp.tile([P, P], bf16, tag="ident")
                make_identity(nc, identity)

                # tri[p, q] = 1.0 where p <= q — causal mask for the one
                # tk==tq diagonal 128x128 sub-block.
                tri = constp.tile([P, P], bf16, tag="tri")
                make_upper_triangular(nc, tri[:], val=1.0, diag=True)

                zeros = constp.tile([P, P], bf16, tag="zeros")
                nc.vector.memset(zeros[:], 0.0)

                QT = qkvp.tile([P, NHC, T], bf16, tag="QT")   # q^T per head [d, t]
                KT = qkvp.tile([P, T], bf16, tag="KT")        # k^T [d, t]
                VT = qkvp.tile([P, T], bf16, tag="VT")        # v^T [d, t]
                VAUG = qkvp.tile([P, TB, HD + 1], bf16, tag="VAUG")  # v [tk, 129]

                # ---- Phase 1: projections. q^T/k^T/v^T = W @ x^T, contracting
                # over e in 16-matmul PSUM chains, two chains per 2-bank tile.
                # K and V first so attention starts as early as possible.
                XT = xqp.tile([P, KO, T], bf16, tag="XT")
                WQT = w1p.tile([P, KO, NHC * HD], bf16, tag="WQT")
                WKT = w1p.tile([P, KO, HD], bf16, tag="WKT")
                WVT = w1p.tile([P, KO, HD], bf16, tag="WVT")

                for ko in range(KO if 1 in phases else 0):
                    nc.sync.dma_start(XT[:, ko], xT_r[:, ko])
                    nc.sync.dma_start(WKT[:, ko], wkT_r[:, ko])
                    nc.sync.dma_start(WVT[:, ko], wvT_r[:, ko])
                    nc.sync.dma_start(WQT[:, ko], wqT_r[:, ko])

                def _acc2(dst, lhsT_of_ko, tcol2):
                    # two 16-matmul chains (tcols 2*tcol2, 2*tcol2+1) into the
                    # two banks of one [P, 1024] PSUM tile, one DVE evacuation
                    ps = ps_acc.tile([P, 1024], f32, tag="ps_acc")
                    for half in range(2):
                        tcol = 2 * tcol2 + half
                        for ko in range(KO):
                            nc.tensor.matmul(
                                ps[:, half * 512:(half + 1) * 512],
                                lhsT_of_ko(ko),
                                XT[:, ko, tcol * 512:(tcol + 1) * 512],
                                start=(ko == 0),
                                stop=(ko == KO - 1),
                            )
                    nc.vector.tensor_copy(dst, ps[:])

                for tcol2 in range(2 if 1 in phases else 0):
                    _acc2(KT[:, tcol2 * 1024:(tcol2 + 1) * 1024],
                          lambda ko: WKT[:, ko], tcol2)
                for tcol2 in range(2 if 1 in phases else 0):
                    _acc2(VT[:, tcol2 * 1024:(tcol2 + 1) * 1024],
                          lambda ko: WVT[:, ko], tcol2)

                # v^T -> v natural layout blocks, with ones column for the
                # softmax denominator.
                if 1 in phases:
                    nc.vector.memset(VAUG[:, :, HD:HD + 1], 1.0)
                for tb in range(TB if 1 in phases else 0):
                    pst = ps_t.tile([P, P], bf16, tag="ps_t")
                    nc.tensor.transpose(
                        pst[:], VT[:, tb * P:(tb + 1) * P], identity[:]
                    )
                    nc.vector.tensor_copy(VAUG[:, tb, 0:HD], pst[:])

                YT = big2.tile([P, NHC, T], bf16, tag="YT")  # y^T per head
                WOT = big2.tile([P, NHC, E], bf16, tag="WOT")
                for ko in range(4 if (2 in phases or 3 in phases) else 0):
                    nc.sync.dma_start(WOT[:, ko], woT_r[:, ko])

                # ---- Phase 2 (interleaved with the per-head Q projection):
                # causal attention on transposed scores. Score blocks for tk
                # pairs land in the two banks of one [P, 1024] PSUM tile so a
                # single Exp covers both; diagonal-region blocks are narrowed
                # to the causally-valid tq columns and only the tk==tq
                # 128x128 sub-block needs the triangular mask.
                for h in range(NHC if 1 in phases else 0):
                    for tcol2 in range(2):
                        _acc2(
                            QT[:, h, tcol2 * 1024:(tcol2 + 1) * 1024],
                            lambda ko, h=h: WQT[:, ko, h * HD:(h + 1) * HD],
                            tcol2,
                        )
                    if 2 not in phases:
                        continue
                    for tqc in range(TQC):
                        ntk = 4 * (tqc + 1)   # tk blocks up to the diagonal
                        # Two psy accumulators per PSUM bank (129 cols each).
                        # The zero-matmul's start=True clears the whole
                        # bank's has_written bits; both chains then
                        # accumulate with start=False (first write of each
                        # element overwrites because its bit is clear).
                        pys = [
                            ps_y.tile([P, 2 * (HD + 1)], f32, tag="ps_y",
                                      name=f"py_{jj}")
                            for jj in range(2)
                        ]
                        for py in pys:
                            nc.tensor.matmul(
                                py[:], zeros[:], KT[:, 0:2 * (HD + 1)],
                                start=True, stop=False,
                            )

                        def psy(j):
                            return pys[j // 2][:, (j % 2) * (HD + 1):
                                               (j % 2 + 1) * (HD + 1)]

                        for m in range(ntk // 2):
                            t0, t1 = 2 * m, 2 * m + 1
                            i0 = t0 - 4 * tqc
                            i1 = t1 - 4 * tqc
                            off0 = max(0, i0) * P
                            off1 = max(0, i1) * P
                            w0 = 512 - off0
                            w1 = 512 - off1
                            pss = ps_acc.tile([P, 1024], f32, tag="ps_acc")
                            nc.tensor.matmul(
                                pss[:, 0:w0],
                                KT[:, t0 * P:(t0 + 1) * P],
                                QT[:, h, tqc * 512 + off0:(tqc + 1) * 512],
                                start=True,
                                stop=True,
                            )
                            nc.tensor.matmul(
                                pss[:, w0:w0 + w1],
                                KT[:, t1 * P:(t1 + 1) * P],
                                QT[:, h, tqc * 512 + off1:(tqc + 1) * 512],
                                start=True,
                                stop=True,
                            )
                            es = work.tile([P, 1024], bf16, tag="expS")
                            nc.scalar.activation(
                                es[:, 0:w0 + w1], pss[:, 0:w0 + w1],
                                mybir.ActivationFunctionType.Exp,
                                scale=SCALE,
                            )
                            if i0 >= 0:
                                nc.vector.tensor_mul(
                                    out=es[:, 0:P], in0=es[:, 0:P], in1=tri[:]
                                )
                            if i1 >= 0:
                                nc.vector.tensor_mul(
                                    out=es[:, w0:w0 + P],
                                    in0=es[:, w0:w0 + P], in1=tri[:],
                                )
                            for j in range(max(0, i0), 4):
                                nc.tensor.matmul(
                                    psy(j),
                                    es[:, j * P - off0:(j + 1) * P - off0],
                                    VAUG[:, t0],
                                    start=False,
                                    stop=(t0 == 4 * tqc + j),
                                )
                            for j in range(max(0, i1), 4):
                                nc.tensor.matmul(
                                    psy(j),
                                    es[:, w0 + j * P - off1:
                                        w0 + (j + 1) * P - off1],
                                    VAUG[:, t1],
                                    start=False,
                                    stop=(t1 == 4 * tqc + j),
                                )
                        for j in range(4):
                            jg = 4 * tqc + j
                            pj = psy(j)
                            recip = work.tile([P, 1], f32, tag="recip")
                            nc.vector.reciprocal(recip[:], pj[:, HD:HD + 1])
                            ysb = work.tile([P, P], bf16, tag="ysb")
                            nc.vector.tensor_scalar_mul(
                                ysb[:], pj[:, 0:HD], recip[:]
                            )
                            pst = ps_t.tile([P, P], bf16, tag="ps_t")
                            nc.tensor.transpose(pst[:], ysb[:], identity[:])
                            nc.vector.tensor_copy(
                                YT[:, h, jg * P:(jg + 1) * P], pst[:]
                            )

                # ---- Phase 3: o_proj partial: out = sum_h y_h^T.T @ woT_h.
                # Two 4-matmul chains per 2-bank PSUM tile; evacuations
                # alternate DVE/ACT; one [128, 2048] bf16 store per tb (4KB
                # contiguous per partition).
                for tb in range(TB if 3 in phases else 0):
                    osb = owork.tile([P, E], bf16, tag="osb")
                    for ec2 in range(2):
                        ps = ps_acc.tile([P, 1024], f32, tag="ps_acc")
                        for half in range(2):
                            ec = 2 * ec2 + half
                            for h in range(NHC):
                                nc.tensor.matmul(
                                    ps[:, half * 512:(half + 1) * 512],
                                    YT[:, h, tb * P:(tb + 1) * P],
                                    WOT[:, h, ec * 512:(ec + 1) * 512],
                                    start=(h == 0),
                                    stop=(h == 3),
                                )
                        dst = osb[:, ec2 * 1024:(ec2 + 1) * 1024]
                        if ec2 == 0:
                            nc.vector.tensor_copy(dst, ps[:])
                        else:
                            nc.scalar.copy(dst, ps[:])
                    nc.sync.dma_start(out_r[:, tb], osb[:])

            if loop_reps == 1:
                body()
            else:
                with tc.For_i(0, loop_reps, 1):
                    body()

    nc.finalize()
    return nc


def _get_nc():
    if "nc" not in _NC_CACHE:
        _NC_CACHE["nc"] = _build_nc()
    return _NC_CACHE["nc"]


def _in_maps(x, wq, wk, wv, wo):
    import ml_dtypes

    bf16 = ml_dtypes.bfloat16
    xTb = [np.ascontiguousarray(x[b].T.astype(bf16)) for b in range(B)]
    wqT = [
        np.ascontiguousarray(wq[g * 512:(g + 1) * 512].T.astype(bf16))
        for g in range(NKV)
    ]
    wkT = [
        np.ascontiguousarray(wk[g * HD:(g + 1) * HD].T.astype(bf16))
        for g in range(NKV)
    ]
    wvT = [
        np.ascontiguousarray(wv[g * HD:(g + 1) * HD].T.astype(bf16))
        for g in range(NKV)
    ]
    woT = [
        np.ascontiguousarray(wo[:, g * 512:(g + 1) * 512].T.astype(bf16))
        for g in range(NKV)
    ]
    maps = []
    for c in range(N_CORES):
        b, g = divmod(c, NKV)
        maps.append({
            "xT": xTb[b],
            "wqT": wqT[g],
            "wkT": wkT[g],
            "wvT": wvT[g],
            "woT": woT[g],
        })
    return maps


def kernel(x, wq, wk, wv, wo):
    from concourse.bass_utils import run_bass_kernel_spmd

    x = np.asarray(x, dtype=np.float32)
    wq = np.asarray(wq, dtype=np.float32)
    wk = np.asarray(wk, dtype=np.float32)
    wv = np.asarray(wv, dtype=np.float32)
    wo = np.asarray(wo, dtype=np.float32)

    nc = _get_nc()
    in_maps = _in_maps(x, wq, wk, wv, wo)

    res = run_bass_kernel_spmd(nc, in_maps, core_ids=list(range(N_CORES)))

    partials = [res.results[c]["out"] for c in range(N_CORES)]
    out = np.empty((B, T, E), dtype=np.float32)
    for b in range(B):
        acc = partials[NKV * b].astype(np.float32)
        for g in range(1, NKV):
            acc = acc + partials[NKV * b + g].astype(np.float32)
        out[b] = acc
    return out


# revision 39
# speedup vs baseline: 4.3065x; 1.4398x over previous
"""GQA attention block (QKV proj + causal attention + output proj) on 8 trn2 cores.

Sharding: core c -> (batch b = c//4, kv-group g = c%4). Each core computes 4 Q
heads (one KV-head group) of one batch and a partial o_proj output; the host
sums the 4 partials per batch (row-sharded o_proj all-reduce done host-side).

All matmul operands are bf16 (PE streams ~2 cols/cycle vs 1 for fp32's
4-cycle rows) with fp32 PSUM accumulation. Attention uses transposed scores
S^T[tk, tq] so the softmax denominator comes for free from a ones-column
appended to V. Score blocks are computed in pairs into 2-bank [128,1024]
PSUM tiles so one Exp activation covers two blocks (halves ACT call count);
the four per-head psy accumulators live two-per-PSUM-bank, opened by a
zero-writing matmul whose start=True clears the whole bank's has_written
bits (values irrelevant: it writes zeros, later matmuls accumulate).
The attention loop runs tqc-outer so each 512-token chunk's o_proj is
emitted right after its four heads finish: the PE-bound o_proj chains
overlap the ACT-bound attention of the next chunk.

_build_nc(loop_reps=N) wraps the body in a For_i so one NEFF execution runs
the kernel N times — used by the bench harness to difference two rep counts
and cancel per-call RPC overhead (axon has no NTFF profiling).
"""

import math

import numpy as np

# Model dims (hardcoded per contract; kernel.py must be self-contained).
B = 2
T = 2048
E = 2048
HD = 128               # head dim
NH = 16                # query heads total
NKV = 4                # kv heads total
NHC = 4                # query heads per core
P = 128
KO = E // P            # 16 contraction subtiles of 128
TQC = T // 512         # 4 query chunks of 512
TB = T // P            # 16 t blocks of 128
SCALE = 1.0 / math.sqrt(HD)
N_CORES = 8

_NC_CACHE = {}


def _build_nc(loop_reps=1, phases=(1, 2, 3)):
    import concourse.bacc as bacc
    import concourse.mybir as mybir
    import concourse.tile as tile
    from concourse.masks import make_identity, make_upper_triangular

    f32 = mybir.dt.float32
    bf16 = mybir.dt.bfloat16
    nc = bacc.Bacc(None, target_bir_lowering=False)

    xT = nc.dram_tensor("xT", [E, T], bf16, kind="ExternalInput")
    wqT = nc.dram_tensor("wqT", [E, NHC * HD], bf16, kind="ExternalInput")
    wkT = nc.dram_tensor("wkT", [E, HD], bf16, kind="ExternalInput")
    wvT = nc.dram_tensor("wvT", [E, HD], bf16, kind="ExternalInput")
    woT = nc.dram_tensor("woT", [NHC * HD, E], bf16, kind="ExternalInput")
    out = nc.dram_tensor("out", [T, E], bf16, kind="ExternalOutput")

    xT_r = xT.rearrange("(ko p) t -> p ko t", p=P)        # [128, 16, T]
    wqT_r = wqT.rearrange("(ko p) d -> p ko d", p=P)      # [128, 16, 512]
    wkT_r = wkT.rearrange("(ko p) d -> p ko d", p=P)      # [128, 16, 128]
    wvT_r = wvT.rearrange("(ko p) d -> p ko d", p=P)
    woT_r = woT.rearrange("(h p) e -> p h e", p=P)        # [128, 4, E]
    out_r = out.rearrange("(tb p) e -> p tb e", p=P)      # [128, 16, E]

    with tile.TileContext(nc) as tc:
        with (
            tc.tile_pool(name="const", bufs=1) as constp,
            tc.tile_pool(name="qkv", bufs=1) as qkvp,
            tc.tile_pool(name="w1", bufs=1) as w1p,
            tc.tile_pool(name="xq", bufs=1) as xqp,
            tc.tile_pool(name="big2", bufs=1) as big2,
            tc.tile_pool(name="work", bufs=4) as work,
            tc.tile_pool(name="owork", bufs=3) as owork,
            # PSUM budget (8 banks): scores 2x2-bank, chains 2x1-bank
            # (transposes borrow chain slots), psy pairs 2x1-bank.
            tc.tile_pool(name="ps_score", bufs=2, space="PSUM") as ps_score,
            tc.tile_pool(name="ps_chain", bufs=2, space="PSUM") as ps_chain,
            tc.tile_pool(name="ps_y", bufs=2, space="PSUM") as ps_y,
        ):
            def body():
                identity = constp.tile([P, P], bf16, tag="ident")
                make_identity(nc, identity)

                # tri[p, q] = 1.0 where p <= q — causal mask for the one
                # tk==tq diagonal 128x128 sub-block.
                tri = constp.tile([P, P], bf16, tag="tri")
                make_upper_triangular(nc, tri[:], val=1.0, diag=True)

                zeros = constp.tile([P, P], bf16, tag="zeros")
                nc.vector.memset(zeros[:], 0.0)

                QT = qkvp.tile([P, NHC, T], bf16, tag="QT")   # q^T per head [d, t]
                KT = qkvp.tile([P, T], bf16, tag="KT")        # k^T [d, t]
                VT = qkvp.tile([P, T], bf16, tag="VT")        # v^T [d, t]
                VAUG = qkvp.tile([P, TB, HD + 1], bf16, tag="VAUG")  # v [tk, 129]

                # ---- Phase 1: projections. q^T/k^T/v^T = W @ x^T, contracting
                # over e in 16-matmul PSUM chains, two chains per 2-bank tile.
                # K and V first so attention starts as early as possible.
                XT = xqp.tile([P, KO, T], bf16, tag="XT")
                WQT = w1p.tile([P, KO, NHC * HD], bf16, tag="WQT")
                WKT = w1p.tile([P, KO, HD], bf16, tag="WKT")
                WVT = w1p.tile([P, KO, HD], bf16, tag="WVT")

                # XT in 32 half-T loads, first halves first, so the earliest
                # projection chains unblock after ~half the x transfer; weight
                # loads batched to cut SP-sequencer DMA issue serialization.
                if 1 in phases:
                    nc.sync.dma_start(WKT[:], wkT_r[:])
                    nc.sync.dma_start(WVT[:], wvT_r[:])
                    for half in range(2):
                        cols = slice(half * 1024, (half + 1) * 1024)
                        for ko in range(KO):
                            nc.sync.dma_start(
                                XT[:, ko, cols], xT_r[:, ko, cols]
                            )
                    for q in range(4):
                        nc.sync.dma_start(
                            WQT[:, 4 * q:4 * (q + 1)],
                            wqT_r[:, 4 * q:4 * (q + 1)],
                        )

                def _acc(dst, lhsT_of_ko, tcol):
                    # 16-matmul PSUM chain, one DVE evacuation
                    ps = ps_chain.tile([P, 512], f32, tag="ps_chain")
                    for ko in range(KO):
                        nc.tensor.matmul(
                            ps[:],
                            lhsT_of_ko(ko),
                            XT[:, ko, tcol * 512:(tcol + 1) * 512],
                            start=(ko == 0),
                            stop=(ko == KO - 1),
                        )
                    nc.vector.tensor_copy(dst, ps[:])

                for tcol in range(TQC if 1 in phases else 0):
                    _acc(KT[:, tcol * 512:(tcol + 1) * 512],
                         lambda ko: WKT[:, ko], tcol)
                for tcol in range(TQC if 1 in phases else 0):
                    _acc(VT[:, tcol * 512:(tcol + 1) * 512],
                         lambda ko: WVT[:, ko], tcol)

                # v^T -> v natural layout blocks, with ones column for the
                # softmax denominator.
                if 1 in phases:
                    nc.vector.memset(VAUG[:, :, HD:HD + 1], 1.0)
                for tb in range(TB if 1 in phases else 0):
                    pst = ps_chain.tile([P, P], bf16, tag="ps_chain")
                    nc.tensor.transpose(
                        pst[:], VT[:, tb * P:(tb + 1) * P], identity[:]
                    )
                    nc.vector.tensor_copy(VAUG[:, tb, 0:HD], pst[:])

                YT = big2.tile([P, NHC, T], bf16, tag="YT")  # y^T per head
                WOT = big2.tile([P, NHC, E], bf16, tag="WOT")
                for ko in range(4 if (2 in phases or 3 in phases) else 0):
                    nc.sync.dma_start(WOT[:, ko], woT_r[:, ko])

                for h in range(NHC if 1 in phases else 0):
                    for tcol in range(TQC):
                        _acc(
                            QT[:, h, tcol * 512:(tcol + 1) * 512],
                            lambda ko, h=h: WQT[:, ko, h * HD:(h + 1) * HD],
                            tcol,
                        )

                # ---- Phase 2: causal attention on transposed scores, tqc
                # outer so each 512-row chunk's o_proj (phase 3) is emitted
                # right after its four heads finish — the PE-bound o_proj
                # chains overlap the ACT-bound attention of the next chunk.
                # Score blocks for tk pairs land in the two banks of one
                # [P, 1024] PSUM tile so a single Exp covers both;
                # diagonal-region blocks are narrowed to the causally-valid
                # tq columns and only the tk==tq 128x128 sub-block needs the
                # triangular mask.
                def attention(h, tqc):
                        ntk = 4 * (tqc + 1)   # tk blocks up to the diagonal
                        # Two psy accumulators per PSUM bank (129 cols each).
                        # The zero-matmul's start=True clears the whole
                        # bank's has_written bits; both chains then
                        # accumulate with start=False (first write of each
                        # element overwrites because its bit is clear).
                        pys = [
                            ps_y.tile([P, 2 * (HD + 1)], f32, tag="ps_y",
                                      name=f"py_{jj}")
                            for jj in range(2)
                        ]
                        for py in pys:
                            nc.tensor.matmul(
                                py[:], zeros[:], KT[:, 0:2 * (HD + 1)],
                                start=True, stop=False,
                            )

                        def psy(j):
                            return pys[j // 2][:, (j % 2) * (HD + 1):
                                               (j % 2 + 1) * (HD + 1)]

                        for m in range(ntk // 2):
                            t0, t1 = 2 * m, 2 * m + 1
                            i0 = t0 - 4 * tqc
                            i1 = t1 - 4 * tqc
                            off0 = max(0, i0) * P
                            off1 = max(0, i1) * P
                            w0 = 512 - off0
                            w1 = 512 - off1
                            pss = ps_score.tile([P, 1024], f32, tag="ps_score")
                            nc.tensor.matmul(
                                pss[:, 0:w0],
                                KT[:, t0 * P:(t0 + 1) * P],
                                QT[:, h, tqc * 512 + off0:(tqc + 1) * 512],
                                start=True,
                                stop=True,
                            )
                            nc.tensor.matmul(
                                pss[:, w0:w0 + w1],
                                KT[:, t1 * P:(t1 + 1) * P],
                                QT[:, h, tqc * 512 + off1:(tqc + 1) * 512],
                                start=True,
                                stop=True,
                            )
                            es = work.tile([P, 1024], bf16, tag="expS")
                            nc.scalar.activation(
                                es[:, 0:w0 + w1], pss[:, 0:w0 + w1],
                                mybir.ActivationFunctionType.Exp,
                                scale=SCALE,
                            )
                            if i0 >= 0:
                                nc.vector.tensor_mul(
                                    out=es[:, 0:P], in0=es[:, 0:P], in1=tri[:]
                                )
                            if i1 >= 0:
                                nc.vector.tensor_mul(
                                    out=es[:, w0:w0 + P],
                                    in0=es[:, w0:w0 + P], in1=tri[:],
                                )
                            for j in range(max(0, i0), 4):
                                nc.tensor.matmul(
                                    psy(j),
                                    es[:, j * P - off0:(j + 1) * P - off0],
                                    VAUG[:, t0],
                                    start=False,
                                    stop=(t0 == 4 * tqc + j),
                                )
                            for j in range(max(0, i1), 4):
                                nc.tensor.matmul(
                                    psy(j),
                                    es[:, w0 + j * P - off1:
                                        w0 + (j + 1) * P - off1],
                                    VAUG[:, t1],
                                    start=False,
                                    stop=(t1 == 4 * tqc + j),
                                )
                        for j in range(4):
                            jg = 4 * tqc + j
                            pj = psy(j)
                            recip = work.tile([P, 1], f32, tag="recip")
                            nc.vector.reciprocal(recip[:], pj[:, HD:HD + 1])
                            ysb = work.tile([P, P], bf16, tag="ysb")
                            nc.vector.tensor_scalar_mul(
                                ysb[:], pj[:, 0:HD], recip[:]
                            )
                            pst = ps_chain.tile([P, P], bf16, tag="ps_chain")
                            nc.tensor.transpose(pst[:], ysb[:], identity[:])
                            nc.vector.tensor_copy(
                                YT[:, h, jg * P:(jg + 1) * P], pst[:]
                            )

                # ---- Phase 3: o_proj partial: out = sum_h y_h^T.T @ woT_h.
                # Evacuations alternate DVE/ACT; one [128, 2048] bf16 store
                # per tb (4KB contiguous per partition).
                def o_proj(tb):
                    osb = owork.tile([P, E], bf16, tag="osb")
                    for ec in range(4):
                        ps = ps_chain.tile([P, 512], f32, tag="ps_chain")
                        for h in range(NHC):
                            nc.tensor.matmul(
                                ps[:],
                                YT[:, h, tb * P:(tb + 1) * P],
                                WOT[:, h, ec * 512:(ec + 1) * 512],
                                start=(h == 0),
                                stop=(h == 3),
                            )
                        # DVE only: these overlap next-chunk attention where
                        # ACT (exp) is the bottleneck engine.
                        nc.vector.tensor_copy(
                            osb[:, ec * 512:(ec + 1) * 512], ps[:]
                        )
                    nc.sync.dma_start(out_r[:, tb], osb[:])

                for tqc in range(TQC if (1 in phases and 2 in phases) else 0):
                    for h in range(NHC):
                        attention(h, tqc)
                    if 3 in phases:
                        for tb in range(4 * tqc, 4 * tqc + 4):
                            o_proj(tb)

            if loop_reps == 1:
                body()
            else:
                with tc.For_i(0, loop_reps, 1):
                    body()

    nc.finalize()
    return nc


def _get_nc():
    if "nc" not in _NC_CACHE:
        _NC_CACHE["nc"] = _build_nc()
    return _NC_CACHE["nc"]


def _in_maps(x, wq, wk, wv, wo):
    import ml_dtypes

    bf16 = ml_dtypes.bfloat16
    xTb = [np.ascontiguousarray(x[b].T.astype(bf16)) for b in range(B)]
    wqT = [
        np.ascontiguousarray(wq[g * 512:(g + 1) * 512].T.astype(bf16))
        for g in range(NKV)
    ]
    wkT = [
        np.ascontiguousarray(wk[g * HD:(g + 1) * HD].T.astype(bf16))
        for g in range(NKV)
    ]
    wvT = [
        np.ascontiguousarray(wv[g * HD:(g + 1) * HD].T.astype(bf16))
        for g in range(NKV)
    ]
    woT = [
        np.ascontiguousarray(wo[:, g * 512:(g + 1) * 512].T.astype(bf16))
        for g in range(NKV)
    ]
    maps = []
    for c in range(N_CORES):
        b, g = divmod(c, NKV)
        maps.append({
            "xT": xTb[b],
            "wqT": wqT[g],
            "wkT": wkT[g],
            "wvT": wvT[g],
            "woT": woT[g],
        })
    return maps


def kernel(x, wq, wk, wv, wo):
    from concourse.bass_utils import run_bass_kernel_spmd

    x = np.asarray(x, dtype=np.float32)
    wq = np.asarray(wq, dtype=np.float32)
    wk = np.asarray(wk, dtype=np.float32)
    wv = np.asarray(wv, dtype=np.float32)
    wo = np.asarray(wo, dtype=np.float32)

    nc = _get_nc()
    in_maps = _in_maps(x, wq, wk, wv, wo)

    res = run_bass_kernel_spmd(nc, in_maps, core_ids=list(range(N_CORES)))

    partials = [res.results[c]["out"] for c in range(N_CORES)]
    out = np.empty((B, T, E), dtype=np.float32)
    for b in range(B):
        acc = partials[NKV * b].astype(np.float32)
        for g in range(1, NKV):
            acc = acc + partials[NKV * b + g].astype(np.float32)
        out[b] = acc
    return out
